# revision 18
# baseline (speedup 1.0000x reference)
"""Cross-attention kernel for Trainium2 (8 NeuronCores, data-parallel over batch).

Per core (one batch b):
  q = Wq @ x; k = Wk @ xs; v = Wv @ xs          (channel mix, c=64 contraction)
  per head d:  S^T[g,h] = k_d q_d^T             (contract w)
               P^T = exp(S^T/8 + BIAS)          (no-max softmax; bias keeps fp16 exp in range)
               O[h,w] = P^T.T @ V_d ; Z[h] = P^T.T @ 1 ; out = O / Z

v2 design notes:
- Projection inputs/weights are cast to bf16 once per chunk so the
  projection matmuls run at 1 PE cycle/row instead of fp32's 4.
- KV and Q projections are fused in one loop so PE/DMA/ACT/DVE/POOL all
  stream continuously (PE ramps to full clock only after ~3us busy).
- Q pair packs chunks (p, p+32) into PE row groups 0-63/64-127 AND column
  groups 0-63/64-127, so one [128,1024] PSUM tile holds both streams and a
  single full-width copy drains it.
- Drains write fp16 staging tiles whose free-dim layout makes every
  subsequent DMA source 2D-contiguous: K/Q regions are w-split
  ([i, 16g, 128w] so dma_start_transpose gets a [64, 2048] source), V region
  is natural order for the HBM roundtrip.
- Attention is software-pipelined across heads: S(d+1) is issued before
  O(d) so the PE never waits on the exp; vh tiles are persistent with the
  ones-column memset once; st/ops PSUM tags use 4 bufs each (8 banks).
- Output is written fp16 (halves out DMA) and upcast on host.
"""

import sys

try:
    import concourse  # noqa: F401
except ImportError:  # pragma: no cover
    sys.path.insert(0, "/opt/trn_rl_repo")

import numpy as np

import concourse.bass as bass  # noqa: F401
from concourse import bacc
import concourse.mybir as mybir
import concourse.tile as tile

F32 = mybir.dt.float32
F32R = mybir.dt.float32r
BF16 = mybir.dt.bfloat16
F16 = mybir.dt.float16

B = 8
C = 64
H = 256
W = 256
W2 = W // 2

TEMP_INV = 1.0 / float(np.sqrt(C))
EXP_BIAS = -5.0

CH = 1024          # spatial columns per chunk (4 image rows)
NCHUNK = H * W // CH   # 64
NP = NCHUNK // 2       # 32 fused iterations


def build_program(debug_dump=False):
    nc = bacc.Bacc("TRN2", target_bir_lowering=False, debug=False)

    x = nc.dram_tensor("x", [C, H, W], F32, kind="ExternalInput")
    xs = nc.dram_tensor("xs", [C, H, W], F32, kind="ExternalInput")
    wqT = nc.dram_tensor("wqT", [C, C], F32, kind="ExternalInput")
    wkvT = nc.dram_tensor("wkvT", [C, 2 * C], F32, kind="ExternalInput")
    out = nc.dram_tensor("out", [C, H, W], F16, kind="ExternalOutput")
    v_dram = nc.dram_tensor("v_dram", [C, H, W], F16, kind="Internal")

    x_flat = x.rearrange("c h w -> c (h w)")
    xs_flat = xs.rearrange("c h w -> c (h w)")
    v_flat = v_dram.rearrange("c h w -> c (h w)")

    with tile.TileContext(nc) as tc:
        with (
            tc.tile_pool(name="consts", bufs=1) as consts,
            tc.tile_pool(name="stage", bufs=1) as stage,
        ):
            # ---- constants (weights duplicated into both partition halves) ----
            wq2f = consts.tile([128, C], F32)
            wkv2f = consts.tile([128, 2 * C], F32)
            for hlf in range(2):
                nc.gpsimd.dma_start(wq2f[hlf * C:(hlf + 1) * C, :], wqT[:])
                nc.gpsimd.dma_start(wkv2f[hlf * C:(hlf + 1) * C, :], wkvT[:])
            wq2 = consts.tile([128, C], BF16)
            wkv2 = consts.tile([128, 2 * C], BF16)
            nc.vector.tensor_copy(out=wq2[:], in_=wq2f[:])
            nc.vector.tensor_copy(out=wkv2[:], in_=wkv2f[:])
            bias_sb = consts.tile([128, 1], F32)
            nc.vector.memset(bias_sb[:], EXP_BIAS)

            # ---- persistent staging (fp16), written by xbar transposes ----
            Kt = [stage.tile([W2, H, C], F16, tag=f"Kt{i}", name=f"Kt{i}") for i in range(2)]
            Qt = [stage.tile([W2, H, C], F16, tag=f"Qt{i}", name=f"Qt{i}") for i in range(2)]

            # =================== fused projection phase ===================
            # xbar sources (ck, cqt, cqb) keep partitions 64-127 DEAD: the
            # transpose DMA's hardware footprint covers all 128 partitions
            # regardless of the AP, so live data there races (seen on HW).
            def bf16_hi(ap):
                # strided bf16 view of an fp32 tile (high halfwords = trunc)
                return ap.bitcast(BF16).rearrange("c (n two) -> c n two", two=2)[:, :, 1]

            with (
                tc.tile_pool(name="inring", bufs=2) as inring,
                tc.tile_pool(name="comb", bufs=2) as comb,
                tc.tile_pool(name="ps_proj", bufs=2, space="PSUM") as ps_proj,
            ):
                cqt = cqb = None
                for pidx in range(NP):
                    # ---------- KV pair: chunks (2p, 2p+1) ----------
                    in_kv = inring.tile([128, CH], F32, tag="inkv", name="in_kv")
                    for hlf in range(2):
                        off = (2 * pidx + hlf) * CH
                        nc.gpsimd.dma_start(
                            in_kv[hlf * C:(hlf + 1) * C, :], xs_flat[:, off:off + CH]
                        )
                    ps_kv = [
                        ps_proj.tile([2 * C, CH], F32, tag="pskv", name="ps_kv",
                                     padded_shape=[128, CH])
                        for _ in range(2)
                    ]
                    in_kv16 = bf16_hi(in_kv)
                    for j in range(CH // 512):
                        for hlf in range(2):
                            nc.tensor.matmul(
                                ps_kv[hlf][:, j * 512:(j + 1) * 512],
                                wkv2[hlf * C:(hlf + 1) * C, :],
                                in_kv16[hlf * C:(hlf + 1) * C, j * 512:(j + 1) * 512],
                                start=True, stop=True,
                            )
                    # drains: K w-split layout, V natural; fresh tiles per pidx
                    ck = comb.tile([C, 2 * 1024], F16, tag="ck", name="ck")
                    cv = comb.tile([C, 2 * CH], F16, tag="cv", name="cv")
                    ck_v = ck.rearrange("c (i sg w) -> c i sg w", i=2, sg=8, w=W2)
                    for hlf in range(2):
                        ps_k = ps_kv[hlf][0:C, :].rearrange(
                            "c (g i w) -> c i g w", g=4, i=2, w=W2
                        )
                        k_dst = ck_v[:, :, hlf * 4:(hlf + 1) * 4, :]
                        v_dst = cv[:, hlf * CH:(hlf + 1) * CH]
                        v_src = ps_kv[hlf][C:2 * C, :]
                        if hlf == 0:
                            nc.scalar.copy(out=k_dst, in_=ps_k[:])
                            nc.vector.tensor_copy(out=v_dst, in_=v_src)
                        else:
                            nc.vector.tensor_copy(out=k_dst, in_=ps_k[:])
                            nc.scalar.copy(out=v_dst, in_=v_src)
                    nc.sync.dma_start(
                        out=v_flat[:, pidx * 2048:(pidx + 1) * 2048], in_=cv[:]
                    )
                    for i in range(2):
                        nc.sync.dma_start_transpose(
                            out=Kt[i][:, pidx * 8:(pidx + 1) * 8, :],
                            in_=ck[:, i * 1024:(i + 1) * 1024].rearrange(
                                "c (sg w) -> c sg w", w=W2
                            ),
                        )

                    # ---------- Q pair: chunks (p, p+32), row+col packed ----------
                    in_q = inring.tile([128, CH], F32, tag="inq", name="in_q")
                    for hlf in range(2):
                        off = (pidx + hlf * NP) * CH
                        nc.gpsimd.dma_start(
                            in_q[hlf * C:(hlf + 1) * C, :], x_flat[:, off:off + CH]
                        )
                    ps_q = ps_proj.tile([128, CH], F32, tag="psq", name="ps_q")
                    in_q16 = bf16_hi(in_q)
                    for j in range(CH // 512):
                        for hlf in range(2):
                            nc.tensor.matmul(
                                ps_q[hlf * C:(hlf + 1) * C, j * 512:(j + 1) * 512],
                                wq2[hlf * C:(hlf + 1) * C, :],
                                in_q16[hlf * C:(hlf + 1) * C, j * 512:(j + 1) * 512],
                                start=True, stop=True,
                            )
                    if pidx % 2 == 0:
                        cqt = comb.tile([C, 2 * 1024], F16, tag="cqt", name="cqt")
                        cqb = comb.tile([C, 2 * 1024], F16, tag="cqb", name="cqb")
                    sub = pidx % 2
                    for h2, cqx in ((0, cqt), (1, cqb)):
                        cq_v = cqx.rearrange("c (i sg w) -> c i sg w", i=2, sg=8, w=W2)
                        src = ps_q[h2 * C:(h2 + 1) * C, :].rearrange(
                            "c (g i w) -> c i g w", g=4, i=2, w=W2
                        )
                        if h2 == 0:
                            nc.scalar.copy(
                                out=cq_v[:, :, sub * 4:(sub + 1) * 4, :], in_=src[:]
                            )
                        else:
                            nc.vector.tensor_copy(
                                out=cq_v[:, :, sub * 4:(sub + 1) * 4, :], in_=src[:]
                            )
                    if pidx % 2 == 1:
                        blk = pidx // 2
                        for i in range(2):
                            for h2, cqx in ((0, cqt), (1, cqb)):
                                nc.sync.dma_start_transpose(
                                    out=Qt[i][:, h2 * 128 + blk * 8:h2 * 128 + blk * 8 + 8, :],
                                    in_=cqx[:, i * 1024:(i + 1) * 1024].rearrange(
                                        "c (sg w) -> c sg w", w=W2
                                    ),
                                )

            if debug_dump:
                ktd = [nc.dram_tensor(f"Ktd{i}", [W2, H, C], F16, kind="ExternalOutput") for i in range(2)]
                qtd = [nc.dram_tensor(f"Qtd{i}", [W2, H, C], F16, kind="ExternalOutput") for i in range(2)]
                for i in range(2):
                    nc.sync.dma_start(out=ktd[i][:], in_=Kt[i][:])
                    nc.sync.dma_start(out=qtd[i][:], in_=Qt[i][:])

            # =================== attention ===================
            with (
                tc.tile_pool(name="attn", bufs=1) as attn,
                tc.tile_pool(name="ps_attn", bufs=4, space="PSUM") as ps_attn,
            ):
                # persistent vh tiles: [parity][gt], ones column set once
                vh = [
                    [
                        attn.tile([128, W + 1], F16, tag=f"vh{par}{gt}", name="vh")
                        for gt in range(2)
                    ]
                    for par in range(2)
                ]
                for par in range(2):
                    for gt in range(2):
                        nc.gpsimd.memset(vh[par][gt][:, W:W + 1], 1.0)

                def load_v(d):
                    for gt in range(2):
                        nc.gpsimd.dma_start(
                            out=vh[d % 2][gt][:, 0:W],
                            in_=v_dram[d, gt * 128:(gt + 1) * 128, :],
                        )

                def s_exp(d):
                    es = []
                    for gt in range(2):
                        st = ps_attn.tile([128, H], F32, tag="st", name="st")
                        for i in range(2):
                            nc.tensor.matmul(
                                st[:],
                                Kt[i][:, gt * 128:(gt + 1) * 128, d],
                                Qt[i][:, :, d],
                                start=(i == 0), stop=(i == 1),
                            )
                        e = attn.tile([128, H], F16, tag="expS", bufs=4, name="expS")
                        nc.scalar.activation(
                            out=e[:], in_=st[:],
                            func=mybir.ActivationFunctionType.Exp,
                            bias=bias_sb[:], scale=TEMP_INV,
                        )
                        es.append(e)
                    return es

                def o_phase(d, es):
                    for hc in range(2):
                        ops = ps_attn.tile([128, W + 1], F32, tag="ops", name="ops")
                        for gt in range(2):
                            nc.tensor.matmul(
                                ops[:],
                                es[gt][:, hc * 128:(hc + 1) * 128],
                                vh[d % 2][gt][:],
                                start=(gt == 0), stop=(gt == 1),
                            )
                        r = attn.tile([128, 1], F32, tag="r", bufs=4, name="r")
                        nc.vector.reciprocal(r[:], ops[:, W:W + 1])
                        osb = attn.tile([128, W], F16, tag="osb", bufs=4, name="osb")
                        nc.vector.tensor_scalar_mul(osb[:], ops[:, 0:W], r[:])
                        nc.sync.dma_start(
                            out=out[d, hc * 128:(hc + 1) * 128, :], in_=osb[:]
                        )

                # software pipeline: S(d+1) issues before O(d)
                load_v(0)
                load_v(1)
                es_cur = s_exp(0)
                for d in range(C):
                    es_next = s_exp(d + 1) if d + 1 < C else None
                    o_phase(d, es_cur)
                    # prefetch v for d+2: must be issued AFTER o_phase(d),
                    # which is the last reader of the parity-(d%2) vh tiles
                    if d + 2 < C:
                        load_v(d + 2)
                    es_cur = es_next

    nc.compile()
    return nc


_NC_CACHE = None


def _get_program():
    global _NC_CACHE
    if _NC_CACHE is None:
        _NC_CACHE = build_program()
    return _NC_CACHE


def kernel(x, x_s, Wq, Wkv):
    from concourse.bass_utils import run_bass_kernel_spmd

    nc = _get_program()
    wqT = np.ascontiguousarray(Wq.T).astype(np.float32)
    wkvT = np.ascontiguousarray(Wkv.T).astype(np.float32)
    in_maps = [
        {
            "x": np.ascontiguousarray(x[b]),
            "xs": np.ascontiguousarray(x_s[b]),
            "wqT": wqT,
            "wkvT": wkvT,
        }
        for b in range(B)
    ]
    res = run_bass_kernel_spmd(nc, in_maps, list(range(B)))
    return np.stack(
        [res.results[i]["out"].astype(np.float32) for i in range(B)], axis=0
    )


# revision 24
# speedup vs baseline: 1.1771x; 1.1771x over previous
"""Cross-attention kernel for Trainium2 (8 NeuronCores, data-parallel over batch).

Per core (one batch b):
  q = Wq @ x; k = Wk @ xs; v = Wv @ xs          (channel mix, c=64 contraction)
  per head d:  S^T[g,h] = k_d q_d^T             (contract w)
               P^T = exp(S^T/8 + BIAS)          (no-max softmax; bias keeps fp16 exp in range)
               O[h,w] = P^T.T @ V_d ; Z[h] = P^T.T @ 1 ; out = O / Z

v3 design notes:
- Inputs are DMA'd as strided 2-byte loads of the fp32 high halfwords
  (= bf16 truncation), so projection matmuls run bf16 at 1 PE cycle/row
  with no cast instructions and no extra SBUF.
- KV and Q projections are fused in one loop; the Q pair packs chunks
  (p, p+32) into PE row groups 0-63/64-127 AND column groups 0-63/64-127
  (one [128,1024] PSUM tile for both streams).
- PSUM drains are single full-width natural-order [128,1024] fp16 copies.
  K/Q staging (Ktc/Qtc) keeps the w-half INTERLEAVED in the middle index
  (g*2+i), which makes every xbar-transpose source a contiguous 2D block:
  one transpose covers both w-halves of a 16-row g-block.
- Full-width drains also remove the xbar footprint hazard: the transpose
  DMA hardware touches all 128 source partitions regardless of the AP, so
  a staging tile must never receive partial-partition-range writes (this
  raced on HW with split layouts).
- Attention is software-pipelined across heads: S(d+1) is issued before
  O(d); vh tiles are persistent with the ones column memset once; st/ops
  PSUM tags use 4 bufs each (8 banks, projection PSUM pool released first).
- Output is written fp16 (halves out DMA) and upcast on host.
"""

import sys

try:
    import concourse  # noqa: F401
except ImportError:  # pragma: no cover
    sys.path.insert(0, "/opt/trn_rl_repo")

import numpy as np

import concourse.bass as bass  # noqa: F401
from concourse import bacc
import concourse.mybir as mybir
import concourse.tile as tile

F32 = mybir.dt.float32
BF16 = mybir.dt.bfloat16
F16 = mybir.dt.float16

B = 8
C = 64
H = 256
W = 256
W2 = W // 2

TEMP_INV = 1.0 / float(np.sqrt(C))
EXP_BIAS = -5.0

CH = 1024          # spatial columns per chunk (4 image rows)
NP = H * W // CH // 2   # 32 fused iterations


def build_program(debug_dump=False):
    nc = bacc.Bacc("TRN2", target_bir_lowering=False, debug=False)

    x = nc.dram_tensor("x", [C, H, W], F32, kind="ExternalInput")
    xs = nc.dram_tensor("xs", [C, H, W], F32, kind="ExternalInput")
    wqT = nc.dram_tensor("wqT", [C, C], F32, kind="ExternalInput")
    wkvT = nc.dram_tensor("wkvT", [C, 2 * C], F32, kind="ExternalInput")
    out = nc.dram_tensor("out", [C, H, W], F16, kind="ExternalOutput")
    v_dram = nc.dram_tensor("v_dram", [C, H, W], F16, kind="Internal")

    x_flat = x.rearrange("c h w -> c (h w)")
    xs_flat = xs.rearrange("c h w -> c (h w)")
    v_flat = v_dram.rearrange("c h w -> c (h w)")

    def bf16_hi(ap):
        # strided bf16 view of an fp32 SBUF tile: the high halfword of each
        # fp32 is its bf16 truncation (little endian)
        return ap.bitcast(BF16).rearrange("c (n two) -> c n two", two=2)[:, :, 1]

    with tile.TileContext(nc) as tc:
        with (
            tc.tile_pool(name="consts", bufs=1) as consts,
            tc.tile_pool(name="stage", bufs=1) as stage,
        ):
            # ---- constants (weights duplicated into both partition halves) ----
            wq2f = consts.tile([128, C], F32)
            wkv2f = consts.tile([128, 2 * C], F32)
            for hlf in range(2):
                nc.gpsimd.dma_start(wq2f[hlf * C:(hlf + 1) * C, :], wqT[:])
                nc.gpsimd.dma_start(wkv2f[hlf * C:(hlf + 1) * C, :], wkvT[:])
            wq2 = consts.tile([128, C], BF16)
            wkv2 = consts.tile([128, 2 * C], BF16)
            nc.vector.tensor_copy(out=wq2[:], in_=wq2f[:])
            nc.vector.tensor_copy(out=wkv2[:], in_=wkv2f[:])
            bias_sb = consts.tile([128, 1], F32)
            nc.vector.memset(bias_sb[:], EXP_BIAS)

            # ---- persistent staging (fp16), written by xbar transposes ----
            # middle index is (spatial*2 + w_half): Ktc[w2, g*2+i, d]
            Ktc = stage.tile([W2, 2 * H, C], F16, tag="Ktc", name="Ktc")
            Qtc = stage.tile([W2, 2 * H, C], F16, tag="Qtc", name="Qtc")

            # =================== fused projection phase ===================
            with (
                tc.tile_pool(name="inring", bufs=3) as inring,
                tc.tile_pool(name="comb", bufs=2) as comb,
                tc.tile_pool(name="ps_proj", bufs=2, space="PSUM") as ps_proj,
            ):
                ckv = cqc = None
                for pidx in range(NP):
                    # ---------- KV pair: chunks (2p, 2p+1) ----------
                    in_kv = inring.tile([128, CH], F32, tag="inkv", name="in_kv")
                    for hlf in range(2):
                        off = (2 * pidx + hlf) * CH
                        nc.gpsimd.dma_start(
                            in_kv[hlf * C:(hlf + 1) * C, :],
                            xs_flat[:, off:off + CH],
                        )
                    in_kv16 = bf16_hi(in_kv)
                    ps_kv = [
                        ps_proj.tile([2 * C, CH], F32, tag="pskv", name="ps_kv",
                                     padded_shape=[128, CH])
                        for _ in range(2)
                    ]
                    for j in range(CH // 512):
                        for hlf in range(2):
                            nc.tensor.matmul(
                                ps_kv[hlf][:, j * 512:(j + 1) * 512],
                                wkv2[hlf * C:(hlf + 1) * C, :],
                                in_kv16[hlf * C:(hlf + 1) * C, j * 512:(j + 1) * 512],
                                start=True, stop=True,
                            )
                    # full-width natural-order drains (K rows 0-63, V rows 64-127)
                    if pidx % 2 == 0:
                        ckv = comb.tile([128, 2 * 2048], F16, tag="ckv", name="ckv")
                    for hlf in range(2):
                        sub = 2 * (pidx % 2) + hlf
                        dst = ckv[:, sub * CH:(sub + 1) * CH]
                        if hlf == 0:
                            nc.scalar.copy(out=dst, in_=ps_kv[hlf][:])
                        else:
                            nc.vector.tensor_copy(out=dst, in_=ps_kv[hlf][:])
                    if pidx % 2 == 1:
                        blk = pidx // 2
                        nc.sync.dma_start(
                            out=v_flat[:, blk * 4096:(blk + 1) * 4096],
                            in_=ckv[C:2 * C, :],
                        )
                        # one xbar: 16 g-rows x 256 w -> Ktc[w2, 32 (g,i), c]
                        nc.sync.dma_start_transpose(
                            out=Ktc[:, blk * 32:(blk + 1) * 32, :],
                            in_=ckv[0:C, :].rearrange("c (r w) -> c r w", w=W2),
                        )

                    # ---------- Q pair: chunks (p, p+32), row+col packed ----------
                    in_q = inring.tile([128, CH], F32, tag="inq", name="in_q")
                    for hlf in range(2):
                        off = (pidx + hlf * NP) * CH
                        nc.gpsimd.dma_start(
                            in_q[hlf * C:(hlf + 1) * C, :],
                            x_flat[:, off:off + CH],
                        )
                    in_q16 = bf16_hi(in_q)
                    ps_q = ps_proj.tile([128, CH], F32, tag="psq", name="ps_q")
                    for j in range(CH // 512):
                        for hlf in range(2):
                            nc.tensor.matmul(
                                ps_q[hlf * C:(hlf + 1) * C, j * 512:(j + 1) * 512],
                                wq2[hlf * C:(hlf + 1) * C, :],
                                in_q16[hlf * C:(hlf + 1) * C, j * 512:(j + 1) * 512],
                                start=True, stop=True,
                            )
                    if pidx % 2 == 0:
                        cqc = comb.tile([128, 2 * CH], F16, tag="cqc", name="cqc")
                    sub = pidx % 2
                    if pidx % 2 == 0:
                        nc.scalar.copy(
                            out=cqc[:, sub * CH:(sub + 1) * CH], in_=ps_q[:]
                        )
                    else:
                        nc.vector.tensor_copy(
                            out=cqc[:, sub * CH:(sub + 1) * CH], in_=ps_q[:]
                        )
                    if pidx % 2 == 1:
                        blk = pidx // 2
                        # top half: h-rows 8b..8b+7 -> middle [16b, 16b+16)
                        # bottom half: h-rows 128+8b.. -> middle [256+16b, ..)
                        for h2 in range(2):
                            nc.sync.dma_start_transpose(
                                out=Qtc[:, h2 * 256 + blk * 16:h2 * 256 + blk * 16 + 16, :],
                                in_=cqc[h2 * C:(h2 + 1) * C, :].rearrange(
                                    "c (r w) -> c r w", w=W2
                                ),
                            )

            if debug_dump:
                ktd = nc.dram_tensor("Ktd", [W2, 2 * H, C], F16, kind="ExternalOutput")
                qtd = nc.dram_tensor("Qtd", [W2, 2 * H, C], F16, kind="ExternalOutput")
                nc.sync.dma_start(out=ktd[:], in_=Ktc[:])
                nc.sync.dma_start(out=qtd[:], in_=Qtc[:])

            # =================== attention ===================
            Ktv = Ktc.rearrange("p (g i) c -> p g i c", i=2)
            Qtv = Qtc.rearrange("p (h i) c -> p h i c", i=2)
            with (
                tc.tile_pool(name="attn", bufs=1) as attn,
                tc.tile_pool(name="ps_attn", bufs=4, space="PSUM") as ps_attn,
            ):
                # persistent vh tiles: [parity][gt], ones column set once
                vh = [
                    [
                        attn.tile([128, W + 1], F16, tag=f"vh{par}{gt}", name="vh")
                        for gt in range(2)
                    ]
                    for par in range(2)
                ]
                for par in range(2):
                    for gt in range(2):
                        nc.gpsimd.memset(vh[par][gt][:, W:W + 1], 1.0)

                def load_v(d):
                    for gt in range(2):
                        nc.gpsimd.dma_start(
                            out=vh[d % 2][gt][:, 0:W],
                            in_=v_dram[d, gt * 128:(gt + 1) * 128, :],
                        )

                def s_exp(d):
                    es = []
                    for gt in range(2):
                        st = ps_attn.tile([128, H], F32, tag="st", name="st")
                        for i in range(2):
                            nc.tensor.matmul(
                                st[:],
                                Ktv[:, gt * 128:(gt + 1) * 128, i, d],
                                Qtv[:, :, i, d],
                                start=(i == 0), stop=(i == 1),
                            )
                        e = attn.tile([128, H], F16, tag="expS", bufs=4, name="expS")
                        nc.scalar.activation(
                            out=e[:], in_=st[:],
                            func=mybir.ActivationFunctionType.Exp,
                            bias=bias_sb[:], scale=TEMP_INV,
                        )
                        es.append(e)
                    return es

                def o_phase(d, es):
                    for hc in range(2):
                        ops = ps_attn.tile([128, W + 1], F32, tag="ops", name="ops")
                        for gt in range(2):
                            nc.tensor.matmul(
                                ops[:],
                                es[gt][:, hc * 128:(hc + 1) * 128],
                                vh[d % 2][gt][:],
                                start=(gt == 0), stop=(gt == 1),
                            )
                        r = attn.tile([128, 1], F32, tag="r", bufs=4, name="r")
                        nc.vector.reciprocal(r[:], ops[:, W:W + 1])
                        osb = attn.tile([128, W], F16, tag="osb", bufs=4, name="osb")
                        nc.vector.tensor_scalar_mul(osb[:], ops[:, 0:W], r[:])
                        nc.sync.dma_start(
                            out=out[d, hc * 128:(hc + 1) * 128, :], in_=osb[:]
                        )

                # software pipeline: S(d+1) issues before O(d)
                load_v(0)
                load_v(1)
                es_cur = s_exp(0)
                for d in range(C):
                    es_next = s_exp(d + 1) if d + 1 < C else None
                    o_phase(d, es_cur)
                    # prefetch v for d+2: must be issued AFTER o_phase(d),
                    # which is the last reader of the parity-(d%2) vh tiles
                    if d + 2 < C:
                        load_v(d + 2)
                    es_cur = es_next

    nc.compile()
    return nc


_NC_CACHE = None


def _get_program():
    global _NC_CACHE
    if _NC_CACHE is None:
        _NC_CACHE = build_program()
    return _NC_CACHE


def kernel(x, x_s, Wq, Wkv):
    from concourse.bass_utils import run_bass_kernel_spmd

    nc = _get_program()
    wqT = np.ascontiguousarray(Wq.T).astype(np.float32)
    wkvT = np.ascontiguousarray(Wkv.T).astype(np.float32)
    in_maps = [
        {
            "x": np.ascontiguousarray(x[b]),
            "xs": np.ascontiguousarray(x_s[b]),
            "wqT": wqT,
            "wkvT": wkvT,
        }
        for b in range(B)
    ]
    res = run_bass_kernel_spmd(nc, in_maps, list(range(B)))
    return np.stack(
        [res.results[i]["out"].astype(np.float32) for i in range(B)], axis=0
    )


# revision 25
# speedup vs baseline: 1.3242x; 1.1250x over previous
"""Cross-attention kernel for Trainium2 (8 NeuronCores, data-parallel over batch).

Per core (one batch b):
  q = Wq @ x; k = Wk @ xs; v = Wv @ xs          (channel mix, c=64 contraction)
  per head d:  S^T[g,h] = k_d q_d^T             (contract w)
               P^T = exp(S^T/8 + BIAS)          (no-max softmax; bias keeps fp16 exp in range)
               O[h,w] = P^T.T @ V_d ; Z[h] = P^T.T @ 1 ; out = O / Z

v4 design notes:
- Projection matmuls read a strided bf16 view of the fp32 input tiles
  (high halfwords = truncation): bf16 rate with no cast instructions.
- CH=512: every projection PSUM tile is exactly one bank, so both psum
  tags run 4 buffers deep (8 banks) and the drain latency per tile
  halves - the PE never waits long on psum recycling.
- KV and Q projections are fused; the Q pair packs chunks (it, it+64)
  into PE row groups 0-63/64-127 AND column groups 0-63/64-127.
- PSUM drains are single full-width natural-order [128,512] fp16 copies.
  K/Q staging (Ktc/Qtc) keeps the w-half INTERLEAVED in the middle index
  (g*2+i): every xbar-transpose source is a contiguous 2D block covering
  both w-halves, and one transpose moves a 32-row block (xbars have a
  ~1.3us fixed cost, so they are batched 8 iterations per call).
- Full-width drains also remove the xbar footprint hazard: the transpose
  DMA hardware touches all 128 source partitions regardless of the AP
  (partial-partition writes to a staging tile raced on HW).
- Attention is software-pipelined across heads: S(d+1) issues before
  O(d); vh tiles are persistent (ones column memset once); st/ops PSUM
  tags use 4 bufs each; the two normalize muls split across DVE and ACT.
- Output is written fp16 (halves out DMA) and upcast on host.
"""

import sys

try:
    import concourse  # noqa: F401
except ImportError:  # pragma: no cover
    sys.path.insert(0, "/opt/trn_rl_repo")

import numpy as np

import concourse.bass as bass  # noqa: F401
from concourse import bacc
import concourse.mybir as mybir
import concourse.tile as tile

F32 = mybir.dt.float32
BF16 = mybir.dt.bfloat16
F16 = mybir.dt.float16

B = 8
C = 64
H = 256
W = 256
W2 = W // 2

TEMP_INV = 1.0 / float(np.sqrt(C))
EXP_BIAS = -5.0

CH = 512           # spatial columns per chunk (2 image rows) = 1 PSUM bank
NP = H * W // CH // 2   # 64 fused iterations
XB = 8             # iterations per xbar block (16 K g-rows, 16 Q h-rows)


def build_program(debug_dump=False):
    nc = bacc.Bacc("TRN2", target_bir_lowering=False, debug=False)

    x = nc.dram_tensor("x", [C, H, W], F32, kind="ExternalInput")
    xs = nc.dram_tensor("xs", [C, H, W], F32, kind="ExternalInput")
    wqT = nc.dram_tensor("wqT", [C, C], F32, kind="ExternalInput")
    wkvT = nc.dram_tensor("wkvT", [C, 2 * C], F32, kind="ExternalInput")
    out = nc.dram_tensor("out", [C, H, W], F16, kind="ExternalOutput")
    v_dram = nc.dram_tensor("v_dram", [C, H, W], F16, kind="Internal")

    x_flat = x.rearrange("c h w -> c (h w)")
    xs_flat = xs.rearrange("c h w -> c (h w)")
    v_flat = v_dram.rearrange("c h w -> c (h w)")

    def bf16_hi(ap):
        # strided bf16 view of an fp32 SBUF tile: the high halfword of each
        # fp32 is its bf16 truncation (little endian)
        return ap.bitcast(BF16).rearrange("c (n two) -> c n two", two=2)[:, :, 1]

    with tile.TileContext(nc) as tc:
        with (
            tc.tile_pool(name="consts", bufs=1) as consts,
            tc.tile_pool(name="stage", bufs=1) as stage,
        ):
            # ---- constants (weights duplicated into both partition halves) ----
            wq2f = consts.tile([128, C], F32)
            wkv2f = consts.tile([128, 2 * C], F32)
            for hlf in range(2):
                nc.gpsimd.dma_start(wq2f[hlf * C:(hlf + 1) * C, :], wqT[:])
                nc.gpsimd.dma_start(wkv2f[hlf * C:(hlf + 1) * C, :], wkvT[:])
            wq2 = consts.tile([128, C], BF16)
            wkv2 = consts.tile([128, 2 * C], BF16)
            nc.vector.tensor_copy(out=wq2[:], in_=wq2f[:])
            nc.vector.tensor_copy(out=wkv2[:], in_=wkv2f[:])
            bias_sb = consts.tile([128, 1], F32)
            nc.vector.memset(bias_sb[:], EXP_BIAS)

            # ---- persistent staging (fp16), written by xbar transposes ----
            # middle index is (spatial*2 + w_half): Ktc[w2, g*2+i, d]
            Ktc = stage.tile([W2, 2 * H, C], F16, tag="Ktc", name="Ktc")
            Qtc = stage.tile([W2, 2 * H, C], F16, tag="Qtc", name="Qtc")

            # =================== fused projection phase ===================
            with (
                tc.tile_pool(name="inring", bufs=4) as inring,
                tc.tile_pool(name="comb", bufs=2) as comb,
                tc.tile_pool(name="ps_proj", bufs=4, space="PSUM") as ps_proj,
            ):
                ckv = cqc = None
                for it in range(NP):
                    sub = it % XB
                    # ---------- KV pair: chunks (2it, 2it+1) ----------
                    in_kv = inring.tile([128, CH], F32, tag="inkv", name="in_kv")
                    for hlf in range(2):
                        off = (2 * it + hlf) * CH
                        nc.gpsimd.dma_start(
                            in_kv[hlf * C:(hlf + 1) * C, :],
                            xs_flat[:, off:off + CH],
                        )
                    in_kv16 = bf16_hi(in_kv)
                    ps_kv = [
                        ps_proj.tile([2 * C, CH], F32, tag="pskv", name="ps_kv",
                                     padded_shape=[128, CH])
                        for _ in range(2)
                    ]
                    for hlf in range(2):
                        nc.tensor.matmul(
                            ps_kv[hlf][:],
                            wkv2[hlf * C:(hlf + 1) * C, :],
                            in_kv16[hlf * C:(hlf + 1) * C, :],
                            start=True, stop=True,
                        )
                    # full-width natural drains (K rows 0-63, V rows 64-127)
                    if sub == 0:
                        ckv = comb.tile([128, 2 * XB * CH], F16, tag="ckv", name="ckv")
                    for hlf in range(2):
                        s2 = 2 * sub + hlf
                        dst = ckv[:, s2 * CH:(s2 + 1) * CH]
                        if hlf == 0:
                            nc.scalar.copy(out=dst, in_=ps_kv[hlf][:])
                        else:
                            nc.vector.tensor_copy(out=dst, in_=ps_kv[hlf][:])
                    if sub == XB - 1:
                        blk = it // XB
                        nc.sync.dma_start(
                            out=v_flat[:, blk * 8192:(blk + 1) * 8192],
                            in_=ckv[C:2 * C, :],
                        )
                        # one xbar: 32 g-rows x 256 w -> Ktc[w2, 64 (g,i), c]
                        nc.sync.dma_start_transpose(
                            out=Ktc[:, blk * 64:(blk + 1) * 64, :],
                            in_=ckv[0:C, :].rearrange("c (r w) -> c r w", w=W2),
                        )

                    # ---------- Q pair: chunks (it, it+64), row+col packed ----------
                    in_q = inring.tile([128, CH], F32, tag="inq", name="in_q")
                    for hlf in range(2):
                        off = (it + hlf * NP) * CH
                        nc.gpsimd.dma_start(
                            in_q[hlf * C:(hlf + 1) * C, :],
                            x_flat[:, off:off + CH],
                        )
                    in_q16 = bf16_hi(in_q)
                    ps_q = ps_proj.tile([128, CH], F32, tag="psq", name="ps_q")
                    for hlf in range(2):
                        nc.tensor.matmul(
                            ps_q[hlf * C:(hlf + 1) * C, :],
                            wq2[hlf * C:(hlf + 1) * C, :],
                            in_q16[hlf * C:(hlf + 1) * C, :],
                            start=True, stop=True,
                        )
                    if sub == 0:
                        cqc = comb.tile([128, XB * CH], F16, tag="cqc", name="cqc")
                    if it % 2 == 0:
                        nc.scalar.copy(
                            out=cqc[:, sub * CH:(sub + 1) * CH], in_=ps_q[:]
                        )
                    else:
                        nc.vector.tensor_copy(
                            out=cqc[:, sub * CH:(sub + 1) * CH], in_=ps_q[:]
                        )
                    if sub == XB - 1:
                        blk = it // XB
                        # top: h-rows 16b..16b+15 -> middle [32b, 32b+32)
                        # bottom: h-rows 128+16b.. -> middle [256+32b, ..)
                        for h2 in range(2):
                            nc.sync.dma_start_transpose(
                                out=Qtc[:, h2 * 256 + blk * 32:h2 * 256 + blk * 32 + 32, :],
                                in_=cqc[h2 * C:(h2 + 1) * C, :].rearrange(
                                    "c (r w) -> c r w", w=W2
                                ),
                            )

            if debug_dump:
                ktd = nc.dram_tensor("Ktd", [W2, 2 * H, C], F16, kind="ExternalOutput")
                qtd = nc.dram_tensor("Qtd", [W2, 2 * H, C], F16, kind="ExternalOutput")
                nc.sync.dma_start(out=ktd[:], in_=Ktc[:])
                nc.sync.dma_start(out=qtd[:], in_=Qtc[:])

            # =================== attention ===================
            Ktv = Ktc.rearrange("p (g i) c -> p g i c", i=2)
            Qtv = Qtc.rearrange("p (h i) c -> p h i c", i=2)
            with (
                tc.tile_pool(name="attn", bufs=1) as attn,
                tc.tile_pool(name="ps_attn", bufs=4, space="PSUM") as ps_attn,
            ):
                # persistent vh tiles: [parity][gt], ones column set once
                vh = [
                    [
                        attn.tile([128, W + 1], F16, tag=f"vh{par}{gt}", name="vh")
                        for gt in range(2)
                    ]
                    for par in range(2)
                ]
                for par in range(2):
                    for gt in range(2):
                        nc.gpsimd.memset(vh[par][gt][:, W:W + 1], 1.0)

                def load_v(d):
                    for gt in range(2):
                        nc.gpsimd.dma_start(
                            out=vh[d % 2][gt][:, 0:W],
                            in_=v_dram[d, gt * 128:(gt + 1) * 128, :],
                        )

                def s_exp(d):
                    es = []
                    for gt in range(2):
                        st = ps_attn.tile([128, H], F32, tag="st", name="st")
                        for i in range(2):
                            nc.tensor.matmul(
                                st[:],
                                Ktv[:, gt * 128:(gt + 1) * 128, i, d],
                                Qtv[:, :, i, d],
                                start=(i == 0), stop=(i == 1),
                            )
                        e = attn.tile([128, H], F16, tag="expS", bufs=4, name="expS")
                        nc.scalar.activation(
                            out=e[:], in_=st[:],
                            func=mybir.ActivationFunctionType.Exp,
                            bias=bias_sb[:], scale=TEMP_INV,
                        )
                        es.append(e)
                    return es

                def o_phase(d, es):
                    for hc in range(2):
                        ops = ps_attn.tile([128, W + 1], F32, tag="ops", name="ops")
                        for gt in range(2):
                            nc.tensor.matmul(
                                ops[:],
                                es[gt][:, hc * 128:(hc + 1) * 128],
                                vh[d % 2][gt][:],
                                start=(gt == 0), stop=(gt == 1),
                            )
                        r = attn.tile([128, 1], F32, tag="r", bufs=4, name="r")
                        nc.vector.reciprocal(r[:], ops[:, W:W + 1])
                        osb = attn.tile([128, W], F16, tag="osb", bufs=4, name="osb")
                        if hc == 0:
                            nc.vector.tensor_scalar_mul(osb[:], ops[:, 0:W], r[:])
                        else:
                            nc.scalar.activation(
                                out=osb[:], in_=ops[:, 0:W],
                                func=mybir.ActivationFunctionType.Copy,
                                scale=r[:],
                            )
                        nc.sync.dma_start(
                            out=out[d, hc * 128:(hc + 1) * 128, :], in_=osb[:]
                        )

                # software pipeline: S(d+1) issues before O(d)
                load_v(0)
                load_v(1)
                es_cur = s_exp(0)
                for d in range(C):
                    es_next = s_exp(d + 1) if d + 1 < C else None
                    o_phase(d, es_cur)
                    # prefetch v for d+2: must be issued AFTER o_phase(d),
                    # which is the last reader of the parity-(d%2) vh tiles
                    if d + 2 < C:
                        load_v(d + 2)
                    es_cur = es_next

    nc.compile()
    return nc


_NC_CACHE = None


def _get_program():
    global _NC_CACHE
    if _NC_CACHE is None:
        _NC_CACHE = build_program()
    return _NC_CACHE


def kernel(x, x_s, Wq, Wkv):
    from concourse.bass_utils import run_bass_kernel_spmd

    nc = _get_program()
    wqT = np.ascontiguousarray(Wq.T).astype(np.float32)
    wkvT = np.ascontiguousarray(Wkv.T).astype(np.float32)
    in_maps = [
        {
            "x": np.ascontiguousarray(x[b]),
            "xs": np.ascontiguousarray(x_s[b]),
            "wqT": wqT,
            "wkvT": wkvT,
        }
        for b in range(B)
    ]
    res = run_bass_kernel_spmd(nc, in_maps, list(range(B)))
    return np.stack(
        [res.results[i]["out"].astype(np.float32) for i in range(B)], axis=0
    )


# revision 27
# speedup vs baseline: 1.4169x; 1.0700x over previous
"""Cross-attention kernel for Trainium2 (8 NeuronCores, data-parallel over batch).

Per core (one batch b):
  q = Wq @ x; k = Wk @ xs; v = Wv @ xs          (channel mix, c=64 contraction)
  per head d:  S^T[g,h] = k_d q_d^T             (contract w)
               P^T = exp(S^T/8 + BIAS)          (no-max softmax; bias keeps fp16 exp in range)
               O[h,w] = P^T.T @ V_d ; Z[h] = P^T.T @ 1 ; out = O / Z

v4 design notes:
- Projection matmuls read a strided bf16 view of the fp32 input tiles
  (high halfwords = truncation): bf16 rate with no cast instructions.
- CH=512: every projection PSUM tile is exactly one bank, so both psum
  tags run 4 buffers deep (8 banks) and the drain latency per tile
  halves - the PE never waits long on psum recycling.
- KV and Q projections are fused; the Q pair packs chunks (it, it+64)
  into PE row groups 0-63/64-127 AND column groups 0-63/64-127.
- PSUM drains are single full-width natural-order [128,512] fp16 copies.
  K/Q staging (Ktc/Qtc) keeps the w-half INTERLEAVED in the middle index
  (g*2+i): every xbar-transpose source is a contiguous 2D block covering
  both w-halves, and one transpose moves a 32-row block (xbars have a
  ~1.3us fixed cost, so they are batched 8 iterations per call).
- Full-width drains also remove the xbar footprint hazard: the transpose
  DMA hardware touches all 128 source partitions regardless of the AP
  (partial-partition writes to a staging tile raced on HW).
- Attention is software-pipelined across heads: S(d+1) issues before
  O(d); vh tiles are persistent (ones column memset once); st/ops PSUM
  tags use 4 bufs each; the two normalize muls split across DVE and ACT.
- Output is written fp16 (halves out DMA) and upcast on host.
"""

import sys

try:
    import concourse  # noqa: F401
except ImportError:  # pragma: no cover
    sys.path.insert(0, "/opt/trn_rl_repo")

import numpy as np

import concourse.bass as bass  # noqa: F401
from concourse import bacc
import concourse.mybir as mybir
import concourse.tile as tile

F32 = mybir.dt.float32
BF16 = mybir.dt.bfloat16
F16 = mybir.dt.float16

B = 8
C = 64
H = 256
W = 256
W2 = W // 2

TEMP_INV = 1.0 / float(np.sqrt(C))
EXP_BIAS = -5.0

CH = 512           # spatial columns per chunk (2 image rows) = 1 PSUM bank
NP = H * W // CH // 2   # 64 fused iterations
XB = 4             # iterations per xbar block (16 K g-rows, 16 Q h-rows)


def build_program(debug_dump=False):
    nc = bacc.Bacc("TRN2", target_bir_lowering=False, debug=False)

    x = nc.dram_tensor("x", [C, H, W], F32, kind="ExternalInput")
    xs = nc.dram_tensor("xs", [C, H, W], F32, kind="ExternalInput")
    wqT = nc.dram_tensor("wqT", [C, C], F32, kind="ExternalInput")
    wkvT = nc.dram_tensor("wkvT", [C, 2 * C], F32, kind="ExternalInput")
    out = nc.dram_tensor("out", [C, H, W], F16, kind="ExternalOutput")
    v_dram = nc.dram_tensor("v_dram", [C, H, W], F16, kind="Internal")

    x_flat = x.rearrange("c h w -> c (h w)")
    xs_flat = xs.rearrange("c h w -> c (h w)")
    v_flat = v_dram.rearrange("c h w -> c (h w)")

    def bf16_hi(ap):
        # strided bf16 view of an fp32 SBUF tile: the high halfword of each
        # fp32 is its bf16 truncation (little endian)
        return ap.bitcast(BF16).rearrange("c (n two) -> c n two", two=2)[:, :, 1]

    with tile.TileContext(nc) as tc:
        with (
            tc.tile_pool(name="consts", bufs=1) as consts,
            tc.tile_pool(name="stage", bufs=1) as stage,
        ):
            # ---- constants (weights duplicated into both partition halves) ----
            wq2f = consts.tile([128, C], F32)
            wkv2f = consts.tile([128, 2 * C], F32)
            for hlf in range(2):
                nc.gpsimd.dma_start(wq2f[hlf * C:(hlf + 1) * C, :], wqT[:])
                nc.gpsimd.dma_start(wkv2f[hlf * C:(hlf + 1) * C, :], wkvT[:])
            wq2 = consts.tile([128, C], BF16)
            wkv2 = consts.tile([128, 2 * C], BF16)
            nc.vector.tensor_copy(out=wq2[:], in_=wq2f[:])
            nc.vector.tensor_copy(out=wkv2[:], in_=wkv2f[:])
            bias_sb = consts.tile([128, 1], F32)
            nc.vector.memset(bias_sb[:], EXP_BIAS)

            # ---- persistent staging (fp16), written by xbar transposes ----
            # middle index is (spatial*2 + w_half): Ktc[w2, g*2+i, d]
            Ktc = stage.tile([W2, 2 * H, C], F16, tag="Ktc", name="Ktc")
            Qtc = stage.tile([W2, 2 * H, C], F16, tag="Qtc", name="Qtc")

            # =================== fused projection phase ===================
            with (
                tc.tile_pool(name="inring", bufs=4) as inring,
                tc.tile_pool(name="comb", bufs=2) as comb,
                tc.tile_pool(name="ps_proj", bufs=4, space="PSUM") as ps_proj,
            ):
                ckv = cqc = None
                for it in range(NP):
                    sub = it % XB
                    blk = it // XB
                    # ---------- KV pair: chunks (2it, 2it+1), one DMA ----------
                    in_kv = inring.tile([C, 2 * CH], F32, tag="inkv", name="in_kv")
                    nc.gpsimd.dma_start(
                        in_kv[:], xs_flat[:, 2 * it * CH:(2 * it + 2) * CH]
                    )
                    in_kv16 = bf16_hi(in_kv)
                    ps_kv = [
                        ps_proj.tile([2 * C, CH], F32, tag="pskv", name="ps_kv",
                                     padded_shape=[128, CH])
                        for _ in range(2)
                    ]
                    for j in range(2):
                        nc.tensor.matmul(
                            ps_kv[j][:],
                            wkv2[0:C, :],
                            in_kv16[:, j * CH:(j + 1) * CH],
                            start=True, stop=True,
                        )
                    # full-width natural drains (K rows 0-63, V rows 64-127)
                    if sub == 0:
                        ckv = comb.tile([128, 2 * XB * CH], F16, tag="ckv", name="ckv")
                    for j in range(2):
                        s2 = 2 * sub + j
                        dst = ckv[:, s2 * CH:(s2 + 1) * CH]
                        if j == 0:
                            nc.scalar.copy(out=dst, in_=ps_kv[j][:])
                        else:
                            nc.vector.tensor_copy(out=dst, in_=ps_kv[j][:])
                    if sub == XB - 1:
                        nc.sync.dma_start(
                            out=v_flat[:, blk * 4096:(blk + 1) * 4096],
                            in_=ckv[C:2 * C, :],
                        )
                        # one xbar: 16 g-rows x 256 w -> Ktc[w2, 32 (g,i), c]
                        nc.scalar.dma_start_transpose(
                            out=Ktc[:, blk * 32:(blk + 1) * 32, :],
                            in_=ckv[0:C, :].rearrange("c (r w) -> c r w", w=W2),
                        )

                    # ---------- Q pair: chunks (2it, 2it+1), one DMA ----------
                    in_q = inring.tile([C, 2 * CH], F32, tag="inq", name="in_q")
                    nc.gpsimd.dma_start(
                        in_q[:], x_flat[:, 2 * it * CH:(2 * it + 2) * CH]
                    )
                    in_q16 = bf16_hi(in_q)
                    ps_q = [
                        ps_proj.tile([C, CH], F32, tag="psq", name="ps_q",
                                     padded_shape=[128, CH])
                        for _ in range(2)
                    ]
                    for j in range(2):
                        nc.tensor.matmul(
                            ps_q[j][:],
                            wq2[0:C, :],
                            in_q16[:, j * CH:(j + 1) * CH],
                            start=True, stop=True,
                        )
                    if sub == 0:
                        cqc = comb.tile([C, 2 * XB * CH], F16, tag="cqc", name="cqc")
                    for j in range(2):
                        s2 = 2 * sub + j
                        dst = cqc[:, s2 * CH:(s2 + 1) * CH]
                        if j == 0:
                            nc.scalar.copy(out=dst, in_=ps_q[j][:])
                        else:
                            nc.vector.tensor_copy(out=dst, in_=ps_q[j][:])
                    if sub == XB - 1:
                        # one xbar: 16 h-rows x 256 w -> Qtc[w2, 32 (h,i), c]
                        nc.sync.dma_start_transpose(
                            out=Qtc[:, blk * 32:(blk + 1) * 32, :],
                            in_=cqc[:, :].rearrange("c (r w) -> c r w", w=W2),
                        )

            if debug_dump:
                ktd = nc.dram_tensor("Ktd", [W2, 2 * H, C], F16, kind="ExternalOutput")
                qtd = nc.dram_tensor("Qtd", [W2, 2 * H, C], F16, kind="ExternalOutput")
                nc.sync.dma_start(out=ktd[:], in_=Ktc[:])
                nc.sync.dma_start(out=qtd[:], in_=Qtc[:])

            # =================== attention ===================
            Ktv = Ktc.rearrange("p (g i) c -> p g i c", i=2)
            Qtv = Qtc.rearrange("p (h i) c -> p h i c", i=2)
            with (
                tc.tile_pool(name="attn", bufs=1) as attn,
                tc.tile_pool(name="ps_attn", bufs=4, space="PSUM") as ps_attn,
            ):
                # persistent vh tiles: [parity][gt], ones column set once
                vh = [
                    [
                        attn.tile([128, W + 1], F16, tag=f"vh{par}{gt}", name="vh")
                        for gt in range(2)
                    ]
                    for par in range(2)
                ]
                for par in range(2):
                    for gt in range(2):
                        nc.gpsimd.memset(vh[par][gt][:, W:W + 1], 1.0)

                def load_v(d):
                    for gt in range(2):
                        nc.gpsimd.dma_start(
                            out=vh[d % 2][gt][:, 0:W],
                            in_=v_dram[d, gt * 128:(gt + 1) * 128, :],
                        )

                def s_exp(d):
                    es = []
                    for gt in range(2):
                        st = ps_attn.tile([128, H], F32, tag="st", name="st")
                        for i in range(2):
                            nc.tensor.matmul(
                                st[:],
                                Ktv[:, gt * 128:(gt + 1) * 128, i, d],
                                Qtv[:, :, i, d],
                                start=(i == 0), stop=(i == 1),
                            )
                        e = attn.tile([128, H], F16, tag="expS", bufs=4, name="expS")
                        nc.scalar.activation(
                            out=e[:], in_=st[:],
                            func=mybir.ActivationFunctionType.Exp,
                            bias=bias_sb[:], scale=TEMP_INV,
                        )
                        es.append(e)
                    return es

                def o_phase(d, es):
                    for hc in range(2):
                        ops = ps_attn.tile([128, W + 1], F32, tag="ops", name="ops")
                        for gt in range(2):
                            nc.tensor.matmul(
                                ops[:],
                                es[gt][:, hc * 128:(hc + 1) * 128],
                                vh[d % 2][gt][:],
                                start=(gt == 0), stop=(gt == 1),
                            )
                        r = attn.tile([128, 1], F32, tag="r", bufs=4, name="r")
                        nc.vector.reciprocal(r[:], ops[:, W:W + 1])
                        osb = attn.tile([128, W], F16, tag="osb", bufs=4, name="osb")
                        if hc == 0:
                            nc.vector.tensor_scalar_mul(osb[:], ops[:, 0:W], r[:])
                        else:
                            nc.scalar.activation(
                                out=osb[:], in_=ops[:, 0:W],
                                func=mybir.ActivationFunctionType.Copy,
                                scale=r[:],
                            )
                        nc.sync.dma_start(
                            out=out[d, hc * 128:(hc + 1) * 128, :], in_=osb[:]
                        )

                # software pipeline: S(d+1) issues before O(d)
                load_v(0)
                load_v(1)
                es_cur = s_exp(0)
                for d in range(C):
                    es_next = s_exp(d + 1) if d + 1 < C else None
                    o_phase(d, es_cur)
                    # prefetch v for d+2: must be issued AFTER o_phase(d),
                    # which is the last reader of the parity-(d%2) vh tiles
                    if d + 2 < C:
                        load_v(d + 2)
                    es_cur = es_next

    nc.compile()
    return nc


_NC_CACHE = None


def _get_program():
    global _NC_CACHE
    if _NC_CACHE is None:
        _NC_CACHE = build_program()
    return _NC_CACHE


def kernel(x, x_s, Wq, Wkv):
    from concourse.bass_utils import run_bass_kernel_spmd

    nc = _get_program()
    wqT = np.ascontiguousarray(Wq.T).astype(np.float32)
    wkvT = np.ascontiguousarray(Wkv.T).astype(np.float32)
    in_maps = [
        {
            "x": np.ascontiguousarray(x[b]),
            "xs": np.ascontiguousarray(x_s[b]),
            "wqT": wqT,
            "wkvT": wkvT,
        }
        for b in range(B)
    ]
    res = run_bass_kernel_spmd(nc, in_maps, list(range(B)))
    return np.stack(
        [res.results[i]["out"].astype(np.float32) for i in range(B)], axis=0
    )
